# revision 11
# baseline (speedup 1.0000x reference)
"""Trainium2 Bass kernel for nn_LongDistanceAttention (GNN message passing).

Strategy (8 NeuronCores, SPMD, node/row sharding). v2:
  Host prep: A cast to fp8 once (A8 natural, AT8 = per-core A.T column
  block = 1-hop mask M0), X pre-transposed (XT full, XTloc per-core),
  W_s augmented with w1 = W_s@r[:H], w2 = W_s@r[H:] columns. This removes
  the on-device A fp8-cast pipeline, the A8 AllGather (125us unoverlapped
  in v1), and all X/W PE transposes.

  Device, all N x N work on transposed layout [j(source) x i(local rows)]:
    - phase 2: Wh_aug rows + s_j scalars in ONE matmul per chunk against
      the augmented weight; s_i row via w1-column matmul on XTloc.
    - stage 1 GAT: exp(lrelu(s_i+s_j)) = max(exp(e), exp(0.2e)) -> two
      ACT exps with per-partition bias, max + mask-mul on DVE;
      (E @ [Wh | 1 | 0]).T accumulated on PE gives numerator and row-sum.
    - k-hop masks: A^k via fp8 DoubleRow matmuls (exact: 0/1 inputs,
      fp32 PSUM accumulation), binarized by ACT Sign. The 2x512 DR
      instruction stream is interleaved into phase-2/stage-1 PE idle
      slots via MaskEmitter so the PE never drains.
    - h (bf16, ones column) and WaT blocks (f32) all-gathered; both
      collectives overlap the mask2 matmul stream.
    - per hop: ek = expS * mask_k (bf16*fp8 on DVE); U.T/Z via PE;
      normalization via broadcast-then-reciprocal (partition-parallel).
  Final: Y.T = W_out.T @ out.T + b_out, output per core [128, 512].
"""

import sys

import numpy as np

sys.path.insert(0, "/opt/trn_rl_repo")

import concourse.bass as bass  # noqa: E402
import concourse.mybir as mybir  # noqa: E402
import concourse.tile as tile  # noqa: E402
from concourse import bacc  # noqa: E402
from concourse.bass_utils import run_bass_kernel_spmd  # noqa: E402
from concourse.masks import make_identity  # noqa: E402

P = 128
N = 4096
NB = N // P            # 32 j-chunks
HID = 256
OUT_DIM = 128
NCORES = 8
LOC = N // NCORES      # 512 local rows per core
LB = LOC // P          # 4 local partition chunks
ALPHA = 0.2

F32 = mybir.dt.float32
F32R = mybir.dt.float32r
BF16 = mybir.dt.bfloat16
FP8 = mybir.dt.float8e4

_CACHE = {}
last_in_maps = None


def build_kernel():
    nc = bacc.Bacc(
        "TRN2",
        target_bir_lowering=False,
        debug=False,
        enable_asserts=False,
        num_devices=NCORES,
    )

    # ---- kernel I/O (host-prepped layouts) ----
    XT_d = nc.dram_tensor("XT", [HID, N], BF16, kind="ExternalInput")
    XTloc_d = nc.dram_tensor("XTloc", [HID, LOC], F32, kind="ExternalInput")
    A8_d = nc.dram_tensor("A8", [N, N], FP8, kind="ExternalInput")
    AT8_d = nc.dram_tensor("AT8", [N, LOC], FP8, kind="ExternalInput")
    Wsa_d = nc.dram_tensor("Ws_aug", [HID, HID + 2], BF16, kind="ExternalInput")
    w12_d = nc.dram_tensor("w12", [HID, 2], F32, kind="ExternalInput")
    Wl_d = nc.dram_tensor("W_l", [HID, HID], F32, kind="ExternalInput")
    Wo_d = nc.dram_tensor("W_out", [HID, OUT_DIM], F32, kind="ExternalInput")
    bo_d = nc.dram_tensor("b_out", [OUT_DIM], F32, kind="ExternalInput")
    out_d = nc.dram_tensor("out", [OUT_DIM, LOC], F32, kind="ExternalOutput")

    # ---- internal DRAM (single gather blob: hnat bf16 ++ WaT bf16) ----
    GATSZ = 263168
    gat_loc = nc.dram_tensor("gat_loc", [GATSZ], BF16)
    gat_all = nc.dram_tensor("gat_all", [NCORES * GATSZ], BF16,
                             addr_space="Shared")

    groups = [list(range(NCORES))]

    with tile.TileContext(nc) as tc:
        with (
            tc.tile_pool(name="const", bufs=1) as cpool,
            tc.tile_pool(name="small", bufs=1) as sm,
            tc.tile_pool(name="maskp", bufs=1) as mp,
            tc.tile_pool(name="wk", bufs=1) as wk,
            tc.tile_pool(name="pp", bufs=1, space="PSUM") as pp,
        ):
            # =========== constants / weights / masks (ACT queue) ===========
            # M0 first (mask stream feeds on it immediately), in 4 chunks.
            M0 = mp.tile([P, NB, LOC], FP8, name="M0")
            at8_r = AT8_d.ap().rearrange("(c p) n -> p c n", p=P)
            nc.scalar.dma_start(M0[:, 0:8], at8_r[:, 0:8])
            XTloc_sb = cpool.tile([P, 2, LOC], F32R)
            nc.scalar.dma_start(
                XTloc_sb[:],
                XTloc_d.ap().rearrange("(k p) n -> p k n", p=P).bitcast(F32R),
            )
            for q in range(1, 4):
                nc.scalar.dma_start(M0[:, 8 * q : 8 * (q + 1)],
                                    at8_r[:, 8 * q : 8 * (q + 1)])
            Ws_sb = cpool.tile([P, 2, HID + 2], BF16)
            nc.scalar.dma_start(
                Ws_sb[:], Wsa_d.ap().rearrange("(k p) m -> p k m", p=P)
            )
            w12_sb = cpool.tile([P, 2, 2], F32R)
            nc.scalar.dma_start(
                w12_sb[:],
                w12_d.ap().rearrange("(k p) m -> p k m", p=P).bitcast(F32R),
            )
            Wl_sb = cpool.tile([P, 2, HID], F32R)
            nc.scalar.dma_start(
                Wl_sb[:], Wl_d.ap().rearrange("(k p) m -> p k m", p=P).bitcast(F32R)
            )
            Wo_sb = cpool.tile([P, 2, OUT_DIM], F32R)
            nc.scalar.dma_start(
                Wo_sb[:], Wo_d.ap().rearrange("(k p) m -> p k m", p=P).bitcast(F32R)
            )
            bo_sb = cpool.tile([P, 1], F32)
            nc.scalar.dma_start(bo_sb[:], bo_d.ap().rearrange("(o p) -> p o", p=P))
            ident = cpool.tile([P, P], F32)
            make_identity(nc, ident)
            ident_r = cpool.tile([P, P], F32R)
            nc.vector.tensor_copy(ident_r[:], ident[:])
            ones_f = cpool.tile([1, P], F32)
            nc.vector.memset(ones_f[:], 1.0)
            ones_row = cpool.tile([1, P], F32R)
            nc.vector.tensor_copy(ones_row[:], ones_f[:])

            # masks (persist across hops)
            M1 = mp.tile([P, NB, LOC], FP8, name="M1")
            M2 = mp.tile([P, NB, LOC], FP8, name="M2")

            # small persistent tiles
            hT = sm.tile([P, 2, LOC], F32R, name="hT")
            hnat = sm.tile([P, LB, HID + 2], BF16, name="hnat")
            outT = sm.tile([P, 2, LOC], F32R, name="outT")
            WaTloc = sm.tile([P, 2, LOC], BF16, name="WaTloc")
            s_nat = sm.tile([P, NB], F32, name="s_nat")
            s2_nat = sm.tile([P, NB], F32, name="s2_nat")
            B_sb = sm.tile([P, LOC], F32, name="B_sb")
            hTb = sm.tile([P, 2, LOC], BF16, name="hTb")

            # =========== mask matmul emitter (A^k via fp8 DR) ===========
            a8_r = A8_d.ap().rearrange("(kq ko p) n -> p ko kq n", p=P, ko=8)

            class MaskEmitter:
                """Emits the A.T @ rhs fp8-DoubleRow stream (512 matmuls)
                in resumable slabs so mask matmuls fill PE gaps in other
                phases. Per mg (16): kq(4) x s(4) x mi(2) = 32 matmuls,
                then binarize the two PSUM tiles ("act" Sign / "dve"
                is_gt) into the out mask columns."""

                def __init__(self, rhs_tile, out_tile, tag, bin_engine):
                    self.rhs = rhs_tile
                    self.out = out_tile
                    self.tag = tag
                    self.bin_engine = bin_engine
                    self.pos = 0          # 0..511
                    self.pms = None
                    self.a8t = None

                def emit(self, n):
                    end = min(self.pos + n, 512)
                    while self.pos < end:
                        idx = self.pos
                        mg, r = divmod(idx, 32)
                        kq, r2 = divmod(r, 8)
                        s, mi = divmod(r2, 2)
                        if r == 0:
                            self.pms = [
                                pp.tile([P, LOC], F32, tag="mask", bufs=2,
                                        name=f"pm{self.tag}0"),
                                pp.tile([P, LOC], F32, tag="maskB", bufs=1,
                                        name=f"pm{self.tag}1"),
                            ]
                        if r2 == 0:
                            # one DMA per (mg, kq): 8 k-chunks x 256 cols
                            self.a8t = wk.tile([P, 8, 2 * P], FP8, tag="a8t",
                                               bufs=6)
                            nc.sync.dma_start(
                                self.a8t[:],
                                a8_r[:, :, kq, 2 * P * mg : 2 * P * (mg + 1)],
                            )
                        nc.tensor.matmul(
                            self.pms[mi][:],
                            self.a8t[:, 2 * s : 2 * s + 2,
                                     mi * P : (mi + 1) * P],
                            self.rhs[:, 8 * kq + 2 * s : 8 * kq + 2 * s + 2, :],
                            start=(kq == 0 and s == 0),
                            stop=(kq == 3 and s == 3),
                            perf_mode=mybir.MatmulPerfMode.DoubleRow,
                        )
                        if r == 31:
                            for m2 in range(2):
                                if self.bin_engine == "act":
                                    nc.scalar.activation(
                                        self.out[:, 2 * mg + m2],
                                        self.pms[m2][:],
                                        mybir.ActivationFunctionType.Sign,
                                    )
                                else:
                                    nc.vector.tensor_scalar(
                                        self.out[:, 2 * mg + m2],
                                        self.pms[m2][:],
                                        0.5,
                                        None,
                                        mybir.AluOpType.is_gt,
                                    )
                        self.pos += 1

            me1 = MaskEmitter(M0, M1, "a", bin_engine="dve")

            # =========== phase 2: Wh_aug + s vectors ===========
            with tc.tile_pool(name="s1pool", bufs=1) as s1pool:
                Wh_aug = s1pool.tile([P, NB, HID + 2], BF16)
                onez = s1pool.tile([P, NB, 2], BF16)
                nc.vector.memset(onez[:, :, 0:1], 1.0)
                nc.vector.memset(onez[:, :, 1:2], 0.0)
                nc.vector.tensor_copy(Wh_aug[:, :, HID : HID + 2], onez[:])

                # s_i row for local nodes: psr = w1.T @ XTloc
                psr = pp.tile([1, LOC], F32, tag="aggz", bufs=1, name="psr")
                for k in range(2):
                    nc.tensor.matmul(
                        psr[:],
                        w12_sb[:, k, 0:1],
                        XTloc_sb[:, k, :],
                        start=(k == 0),
                        stop=(k == 1),
                    )
                sir = s1pool.tile([1, LOC], F32R)
                nc.vector.tensor_copy(sir[:], psr[:])
                pb = pp.tile([P, LOC], F32, tag="pa", bufs=2, name="pb")
                nc.tensor.matmul(pb[:], ones_row[:], sir[:], start=True,
                                 stop=True)
                nc.vector.tensor_copy(B_sb[:], pb[:])
                me1.emit(32)

                for o in range(NB):
                    xtc = wk.tile([P, 2, P], BF16, tag="xw", bufs=12)
                    nc.scalar.dma_start(
                        xtc[:],
                        XT_d.ap()
                        .rearrange("(k p) n -> p k n", p=P)[:, :, o * P : (o + 1) * P],
                    )
                    pa = pp.tile([P, HID + 2], F32, tag="pa", bufs=2, name="pa")
                    for k in range(2):
                        nc.tensor.matmul(
                            pa[:],
                            xtc[:, k, :],
                            Ws_sb[:, k, :],
                            start=(k == 0),
                            stop=(k == 1),
                        )
                    nc.vector.tensor_copy(Wh_aug[:, o, :HID], pa[:, :HID])
                    nc.vector.tensor_copy(s_nat[:, o : o + 1], pa[:, HID + 1 :])
                    me1.emit(2)
                nc.vector.tensor_scalar(
                    s2_nat[:], s_nat[:], ALPHA, None, mybir.AluOpType.mult
                )

                # =========== phase 3: stage-1 attention ===========
                u0 = pp.tile([P, LOC], F32, tag="agg", bufs=2, name="u0")
                u1 = pp.tile([P, LOC], F32, tag="agg", bufs=2, name="u1")
                uz = pp.tile([2, LOC], F32, tag="aggz", bufs=1, name="uz")
                for jc in range(NB):
                    # exp(lrelu(e)) = max(exp(e), exp(alpha*e)) on ACT
                    e1 = wk.tile([P, LOC], BF16, tag="s1", bufs=8)
                    nc.scalar.activation(
                        e1[:], B_sb[:], mybir.ActivationFunctionType.Exp,
                        bias=s_nat[:, jc : jc + 1],
                    )
                    e2 = wk.tile([P, LOC], BF16, tag="s1", bufs=8)
                    nc.scalar.activation(
                        e2[:], B_sb[:], mybir.ActivationFunctionType.Exp,
                        bias=s2_nat[:, jc : jc + 1], scale=ALPHA,
                    )
                    mx = wk.tile([P, LOC], BF16, tag="s1", bufs=8)
                    nc.vector.tensor_max(out=mx[:], in0=e1[:], in1=e2[:])
                    em = wk.tile([P, LOC], BF16, tag="s1", bufs=8)
                    nc.vector.tensor_mul(out=em[:], in0=mx[:], in1=M0[:, jc])
                    last = jc == NB - 1
                    nc.tensor.matmul(
                        u0[:], Wh_aug[:, jc, 0:P], em[:],
                        start=(jc == 0), stop=last,
                    )
                    nc.tensor.matmul(
                        u1[:], Wh_aug[:, jc, P : 2 * P], em[:],
                        start=(jc == 0), stop=last,
                    )
                    nc.tensor.matmul(
                        uz[:], Wh_aug[:, jc, HID : HID + 2], em[:],
                        start=(jc == 0), stop=last,
                    )

                me1.emit(48)
                # normalize + gelu -> h_local.T [256, 512]
                zrow = s1pool.tile([1, LOC], F32R)
                nc.vector.tensor_copy(zrow[:], uz[0:1, :])
                zbp = pp.tile([P, LOC], F32, tag="pa", bufs=2, name="zbp")
                nc.tensor.matmul(zbp[:], ones_row[:], zrow[:], start=True,
                                 stop=True)
                zr = s1pool.tile([P, LOC], F32)
                nc.vector.reciprocal_approx_fast(out=zr[:], in_=zbp[:])
                for mt, um in enumerate((u0, u1)):
                    tnorm = wk.tile([P, LOC], F32, tag="nrm", bufs=3)
                    nc.vector.tensor_mul(out=tnorm[:], in0=um[:], in1=zr[:])
                    nc.scalar.activation(
                        hT[:, mt], tnorm[:], mybir.ActivationFunctionType.Gelu
                    )
                    nc.vector.tensor_copy(hTb[:, mt], hT[:, mt])

            # =========== phase 4: h transposes + gathers + WaT ===========
            nc.vector.memset(hnat[:, :, HID : HID + 1], 1.0)
            nc.vector.memset(hnat[:, :, HID + 1 : HID + 2], 0.0)
            for ic in range(LB):
                for fc in range(2):
                    pht = pp.tile([P, P], F32R, tag="pa", bufs=2, name="pht")
                    nc.tensor.transpose(
                        pht[:], hT[:, fc, ic * P : (ic + 1) * P], ident_r[:]
                    )
                    nc.vector.tensor_copy(hnat[:, ic, fc * P : (fc + 1) * P],
                                          pht[:])
            nc.scalar.dma_start(
                gat_loc.ap()[0 : LOC * (HID + 2)]
                .rearrange("(c p f) -> p c f", p=P, f=HID + 2),
                hnat[:],
            )
            # local Wa.T block = W_l.T @ h_local.T
            for m2 in range(2):
                pwa = pp.tile([P, LOC], F32, tag="pa", bufs=2, name="pwa")
                for f in range(2):
                    nc.tensor.matmul(
                        pwa[:],
                        Wl_sb[:, f, m2 * P : (m2 + 1) * P],
                        hT[:, f, :],
                        start=(f == 0),
                        stop=(f == 1),
                    )
                nc.vector.tensor_copy(WaTloc[:, m2], pwa[:])
            nc.scalar.dma_start(
                gat_loc.ap()[LOC * (HID + 2) : GATSZ]
                .rearrange("(k p n) -> p k n", p=P, n=LOC),
                WaTloc[:],
            )
            nc.gpsimd.collective_compute(
                "AllGather",
                mybir.AluOpType.bypass,
                ins=[gat_loc[:]],
                outs=[gat_all[:]],
                replica_groups=groups,
            )

            # finish mask1, then mask2 (collectives overlap this stream)
            me1.emit(512)
            me2 = MaskEmitter(M1, M2, "b", bin_engine="dve")
            me2.emit(512)

            with tc.tile_pool(name="hpool", bufs=1) as hp:
                h_aug = hp.tile([P, NB, HID + 2], BF16, name="h_aug")
                for c in range(NCORES):
                    nc.scalar.dma_start(
                        h_aug[:, LB * c : LB * (c + 1)],
                        gat_all.ap()[c * GATSZ : c * GATSZ + LOC * (HID + 2)]
                        .rearrange("(c2 p f) -> p c2 f", p=P, f=HID + 2),
                    )
                expS = hp.tile([P, NB, LOC], BF16, name="expS")

                # ---- scores + expS (needs WaT gather) ----
                with tc.tile_pool(name="scpool", bufs=1) as scpool:
                    WaTall = scpool.tile([P, 2 * NCORES, LOC], BF16)
                    for c in range(NCORES):
                        nc.scalar.dma_start(
                            WaTall[:, 2 * c : 2 * (c + 1)],
                            gat_all.ap()[c * GATSZ + LOC * (HID + 2)
                                         : (c + 1) * GATSZ]
                            .rearrange("(k p n) -> p k n", p=P, n=LOC),
                        )
                    for m in range(NB):
                        pst = pp.tile([P, LOC], F32, tag="pa", bufs=2, name="pst")
                        c, mi = divmod(m, LB)
                        for f in range(2):
                            nc.tensor.matmul(
                                pst[:],
                                WaTall[:, 2 * c + f, mi * P : (mi + 1) * P],
                                hTb[:, f, :],
                                start=(f == 0),
                                stop=(f == 1),
                            )
                        nc.scalar.activation(
                            expS[:, m], pst[:], mybir.ActivationFunctionType.Exp
                        )

                # =========== hops ===========
                def hop(mask_fp8, first, tags=("agg", "aggz")):
                    u0h = pp.tile([P, LOC], F32, tag=tags[0], bufs=2, name="u0h")
                    u1h = pp.tile([P, LOC], F32, tag=tags[0], bufs=2, name="u1h")
                    uzh = pp.tile([2, LOC], F32, tag=tags[1], bufs=1, name="uzh")
                    for m in range(NB):
                        ek = wk.tile([P, LOC], BF16, tag="ek", bufs=6)
                        nc.vector.tensor_mul(
                            out=ek[:], in0=expS[:, m], in1=mask_fp8[:, m]
                        )
                        last = m == NB - 1
                        nc.tensor.matmul(
                            u0h[:], h_aug[:, m, 0:P], ek[:],
                            start=(m == 0), stop=last,
                        )
                        nc.tensor.matmul(
                            u1h[:], h_aug[:, m, P : 2 * P], ek[:],
                            start=(m == 0), stop=last,
                        )
                        nc.tensor.matmul(
                            uzh[:], h_aug[:, m, HID : HID + 2], ek[:],
                            start=(m == 0), stop=last,
                        )
                    zrowh = wk.tile([1, LOC], F32R, tag="row", bufs=2)
                    nc.vector.tensor_copy(zrowh[:], uzh[0:1, :])
                    zbp = pp.tile([P, LOC], F32, tag=tags[1], bufs=1,
                                  name="zbph")
                    nc.tensor.matmul(zbp[:], ones_row[:], zrowh[:],
                                     start=True, stop=True)
                    zrh = wk.tile([P, LOC], F32, tag="nrm", bufs=3)
                    nc.vector.reciprocal_approx_fast(out=zrh[:], in_=zbp[:])
                    for mt, um in enumerate((u0h, u1h)):
                        tn = wk.tile([P, LOC], F32R, tag="nrm", bufs=3)
                        nc.vector.tensor_mul(out=tn[:], in0=um[:], in1=zrh[:])
                        if first:
                            nc.vector.tensor_add(
                                out=outT[:, mt], in0=hT[:, mt], in1=tn[:]
                            )
                        else:
                            nc.vector.tensor_add(
                                out=outT[:, mt], in0=outT[:, mt], in1=tn[:]
                            )

                hop(M0, first=True)
                hop(M1, first=False, tags=("pa", "maskB"))
                hop(M2, first=False)

            # =========== output projection ===========
            py = pp.tile([P, LOC], F32, tag="pa", bufs=2, name="py")
            for k in range(2):
                nc.tensor.matmul(
                    py[:],
                    Wo_sb[:, k, :],
                    outT[:, k, :],
                    start=(k == 0),
                    stop=(k == 1),
                )
            yt = sm.tile([P, LOC], F32, name="yt")
            nc.vector.tensor_scalar(
                yt[:], py[:], bo_sb[:, 0:1], None, mybir.AluOpType.add
            )
            nc.scalar.dma_start(out_d[:, :], yt[:])

    nc.compile()
    return nc


def _get_nc():
    if "nc" not in _CACHE:
        _CACHE["nc"] = build_kernel()
    return _CACHE["nc"]


def kernel(X, A, W_s, r, W_l, W_out, b_out):
    global last_in_maps
    import ml_dtypes

    FP8NP = ml_dtypes.float8_e4m3

    X = np.ascontiguousarray(X, dtype=np.float32)
    A = np.ascontiguousarray(A, dtype=np.float32)
    W_s = np.ascontiguousarray(W_s, dtype=np.float32)
    r = np.ascontiguousarray(r, dtype=np.float32)

    import ml_dtypes as _mld

    XTf = np.ascontiguousarray(X.T)                      # [HID, N] f32
    XT = XTf.astype(_mld.bfloat16)                       # [HID, N] bf16
    A8 = A.astype(FP8NP)                                 # [N, N] (0/1, exact)
    AT8 = np.ascontiguousarray(A8.T)                     # [N, N]
    w1 = W_s @ r[:HID]                                   # [HID, 1]
    w2 = W_s @ r[HID:]                                   # [HID, 1]
    w12 = np.ascontiguousarray(
        np.concatenate([w1, w2], axis=1), dtype=np.float32
    )                                                    # [HID, 2]
    Ws_aug = np.ascontiguousarray(
        np.concatenate([W_s, w1, w2], axis=1)
    ).astype(_mld.bfloat16)                              # [HID, HID+2] bf16

    in_maps = []
    for c in range(NCORES):
        sl = slice(c * LOC, (c + 1) * LOC)
        in_maps.append(
            {
                "XT": XT,
                "XTloc": np.ascontiguousarray(XTf[:, sl]),
                "A8": A8,
                "AT8": np.ascontiguousarray(AT8[:, sl]),
                "Ws_aug": Ws_aug,
                "w12": w12,
                "W_l": np.ascontiguousarray(W_l, dtype=np.float32),
                "W_out": np.ascontiguousarray(W_out, dtype=np.float32),
                "b_out": np.ascontiguousarray(b_out, dtype=np.float32),
            }
        )
    last_in_maps = in_maps
    nc = _get_nc()
    res = run_bass_kernel_spmd(nc, in_maps, core_ids=list(range(NCORES)))
    Y = np.empty((N, OUT_DIM), dtype=np.float32)
    for c in range(NCORES):
        Y[c * LOC : (c + 1) * LOC, :] = res.results[c]["out"].T
    return Y


if __name__ == "__main__":
    build_kernel()
    print("build OK")


# revision 12
# speedup vs baseline: 1.1275x; 1.1275x over previous
"""Trainium2 Bass kernel for nn_LongDistanceAttention (GNN message passing).

Strategy (8 NeuronCores, SPMD, node/row sharding). v2:
  Host prep: A cast to fp8 once (A8 natural, AT8 = per-core A.T column
  block = 1-hop mask M0), X pre-transposed (XT full, XTloc per-core),
  W_s augmented with w1 = W_s@r[:H], w2 = W_s@r[H:] columns. This removes
  the on-device A fp8-cast pipeline, the A8 AllGather (125us unoverlapped
  in v1), and all X/W PE transposes.

  Device, all N x N work on transposed layout [j(source) x i(local rows)]:
    - phase 2: Wh_aug rows + s_j scalars in ONE matmul per chunk against
      the augmented weight; s_i row via w1-column matmul on XTloc.
    - stage 1 GAT: exp(lrelu(s_i+s_j)) = max(exp(e), exp(0.2e)) -> two
      ACT exps with per-partition bias, max + mask-mul on DVE;
      (E @ [Wh | 1 | 0]).T accumulated on PE gives numerator and row-sum.
    - k-hop masks: A^k via fp8 DoubleRow matmuls (exact: 0/1 inputs,
      fp32 PSUM accumulation), binarized by ACT Sign. The 2x512 DR
      instruction stream is interleaved into phase-2/stage-1 PE idle
      slots via MaskEmitter so the PE never drains.
    - h (bf16, ones column) and WaT blocks (f32) all-gathered; both
      collectives overlap the mask2 matmul stream.
    - per hop: ek = expS * mask_k (bf16*fp8 on DVE); U.T/Z via PE;
      normalization via broadcast-then-reciprocal (partition-parallel).
  Final: Y.T = W_out.T @ out.T + b_out, output per core [128, 512].
"""

import sys

import numpy as np

sys.path.insert(0, "/opt/trn_rl_repo")

import concourse.bass as bass  # noqa: E402
import concourse.mybir as mybir  # noqa: E402
import concourse.tile as tile  # noqa: E402
from concourse import bacc  # noqa: E402
from concourse.bass_utils import run_bass_kernel_spmd  # noqa: E402
from concourse.masks import make_identity  # noqa: E402

P = 128
N = 4096
NB = N // P            # 32 j-chunks
HID = 256
OUT_DIM = 128
NCORES = 8
LOC = N // NCORES      # 512 local rows per core
LB = LOC // P          # 4 local partition chunks
ALPHA = 0.2

F32 = mybir.dt.float32
F32R = mybir.dt.float32r
BF16 = mybir.dt.bfloat16
FP8 = mybir.dt.float8e4

_CACHE = {}
last_in_maps = None


def build_kernel():
    nc = bacc.Bacc(
        "TRN2",
        target_bir_lowering=False,
        debug=False,
        enable_asserts=False,
        num_devices=NCORES,
    )

    # ---- kernel I/O (host-prepped layouts) ----
    XT_d = nc.dram_tensor("XT", [HID, N], BF16, kind="ExternalInput")
    XTloc_d = nc.dram_tensor("XTloc", [HID, LOC], F32, kind="ExternalInput")
    A8_d = nc.dram_tensor("A8", [N, N], FP8, kind="ExternalInput")
    AT8_d = nc.dram_tensor("AT8", [N, LOC], FP8, kind="ExternalInput")
    Wsa_d = nc.dram_tensor("Ws_aug", [HID, HID + 2], BF16, kind="ExternalInput")
    w12_d = nc.dram_tensor("w12", [HID, 2], F32, kind="ExternalInput")
    Wl_d = nc.dram_tensor("W_l", [HID, HID], F32, kind="ExternalInput")
    Wo_d = nc.dram_tensor("W_out", [HID, OUT_DIM], F32, kind="ExternalInput")
    bo_d = nc.dram_tensor("b_out", [OUT_DIM], F32, kind="ExternalInput")
    out_d = nc.dram_tensor("out", [OUT_DIM, LOC], F32, kind="ExternalOutput")

    # ---- internal DRAM (single gather blob: hnat bf16 ++ WaT bf16) ----
    GATSZ = 263168
    gat_loc = nc.dram_tensor("gat_loc", [GATSZ], BF16)
    gat_all = nc.dram_tensor("gat_all", [NCORES * GATSZ], BF16,
                             addr_space="Shared")

    groups = [list(range(NCORES))]

    with tile.TileContext(nc) as tc:
        with (
            tc.tile_pool(name="const", bufs=1) as cpool,
            tc.tile_pool(name="small", bufs=1) as sm,
            tc.tile_pool(name="maskp", bufs=1) as mp,
            tc.tile_pool(name="wk", bufs=1) as wk,
            tc.tile_pool(name="pp", bufs=1, space="PSUM") as pp,
        ):
            # =========== constants / weights / masks (ACT queue) ===========
            # M0 first (mask stream feeds on it immediately), in 4 chunks.
            M0 = mp.tile([P, NB, LOC], FP8, name="M0")
            at8_r = AT8_d.ap().rearrange("(c p) n -> p c n", p=P)
            nc.scalar.dma_start(M0[:, 0:8], at8_r[:, 0:8])
            XTloc_sb = cpool.tile([P, 2, LOC], F32R)
            nc.scalar.dma_start(
                XTloc_sb[:],
                XTloc_d.ap().rearrange("(k p) n -> p k n", p=P).bitcast(F32R),
            )
            for q in range(1, 4):
                nc.scalar.dma_start(M0[:, 8 * q : 8 * (q + 1)],
                                    at8_r[:, 8 * q : 8 * (q + 1)])
            Ws_sb = cpool.tile([P, 2, HID + 2], BF16)
            nc.scalar.dma_start(
                Ws_sb[:], Wsa_d.ap().rearrange("(k p) m -> p k m", p=P)
            )
            w12_sb = cpool.tile([P, 2, 2], F32R)
            nc.scalar.dma_start(
                w12_sb[:],
                w12_d.ap().rearrange("(k p) m -> p k m", p=P).bitcast(F32R),
            )
            Wl_sb = cpool.tile([P, 2, HID], F32R)
            nc.scalar.dma_start(
                Wl_sb[:], Wl_d.ap().rearrange("(k p) m -> p k m", p=P).bitcast(F32R)
            )
            Wo_sb = cpool.tile([P, 2, OUT_DIM], F32R)
            nc.scalar.dma_start(
                Wo_sb[:], Wo_d.ap().rearrange("(k p) m -> p k m", p=P).bitcast(F32R)
            )
            bo_sb = cpool.tile([P, 1], F32)
            nc.scalar.dma_start(bo_sb[:], bo_d.ap().rearrange("(o p) -> p o", p=P))
            ident = cpool.tile([P, P], F32)
            make_identity(nc, ident)
            ident_r = cpool.tile([P, P], F32R)
            nc.vector.tensor_copy(ident_r[:], ident[:])
            ones_f = cpool.tile([1, P], F32)
            nc.vector.memset(ones_f[:], 1.0)
            ones_row = cpool.tile([1, P], F32R)
            nc.vector.tensor_copy(ones_row[:], ones_f[:])

            # masks (persist across hops)
            M1 = mp.tile([P, NB, LOC], FP8, name="M1")
            M2 = mp.tile([P, NB, LOC], FP8, name="M2")

            # small persistent tiles
            hT = sm.tile([P, 2, LOC], F32R, name="hT")
            hnat = sm.tile([P, LB, HID + 2], BF16, name="hnat")
            outT = sm.tile([P, 2, LOC], F32R, name="outT")
            WaTloc = sm.tile([P, 2, LOC], BF16, name="WaTloc")
            s_nat = sm.tile([P, NB], F32, name="s_nat")
            s2_nat = sm.tile([P, NB], F32, name="s2_nat")
            B_sb = sm.tile([P, LOC], F32, name="B_sb")
            hTb = sm.tile([P, 2, LOC], BF16, name="hTb")

            # =========== mask matmul emitter (A^k via fp8 DR) ===========
            a8_r = A8_d.ap().rearrange("(kq ko p) n -> p ko kq n", p=P, ko=8)

            class MaskEmitter:
                """Emits the A.T @ rhs fp8-DoubleRow stream (512 matmuls)
                in resumable slabs so mask matmuls fill PE gaps in other
                phases. Per mg (16): kq(4) x s(4) x mi(2) = 32 matmuls,
                then binarize the two PSUM tiles ("act" Sign / "dve"
                is_gt) into the out mask columns."""

                def __init__(self, rhs_tile, out_tile, tag, bin_engine):
                    self.rhs = rhs_tile
                    self.out = out_tile
                    self.tag = tag
                    self.bin_engine = bin_engine
                    self.pos = 0          # 0..511
                    self.pms = None
                    self.a8t = None

                def emit(self, n):
                    end = min(self.pos + n, 512)
                    while self.pos < end:
                        idx = self.pos
                        mg, r = divmod(idx, 32)
                        kq, r2 = divmod(r, 8)
                        s, mi = divmod(r2, 2)
                        if r == 0:
                            self.pms = [
                                pp.tile([P, LOC], F32, tag="mask", bufs=2,
                                        name=f"pm{self.tag}0"),
                                pp.tile([P, LOC], F32, tag="maskB", bufs=1,
                                        name=f"pm{self.tag}1"),
                            ]
                        if r2 == 0:
                            # one DMA per (mg, kq): 8 k-chunks x 256 cols
                            self.a8t = wk.tile([P, 8, 2 * P], FP8, tag="a8t",
                                               bufs=6)
                            nc.sync.dma_start(
                                self.a8t[:],
                                a8_r[:, :, kq, 2 * P * mg : 2 * P * (mg + 1)],
                            )
                        nc.tensor.matmul(
                            self.pms[mi][:],
                            self.a8t[:, 2 * s : 2 * s + 2,
                                     mi * P : (mi + 1) * P],
                            self.rhs[:, 8 * kq + 2 * s : 8 * kq + 2 * s + 2, :],
                            start=(kq == 0 and s == 0),
                            stop=(kq == 3 and s == 3),
                            perf_mode=mybir.MatmulPerfMode.DoubleRow,
                        )
                        if r == 31:
                            for m2 in range(2):
                                if self.bin_engine == "act":
                                    nc.scalar.activation(
                                        self.out[:, 2 * mg + m2],
                                        self.pms[m2][:],
                                        mybir.ActivationFunctionType.Sign,
                                    )
                                else:
                                    nc.vector.tensor_scalar(
                                        self.out[:, 2 * mg + m2],
                                        self.pms[m2][:],
                                        0.5,
                                        None,
                                        mybir.AluOpType.is_gt,
                                    )
                        self.pos += 1

            me1 = MaskEmitter(M0, M1, "a", bin_engine="dve")

            # =========== phase 2: Wh_aug + s vectors ===========
            with tc.tile_pool(name="s1pool", bufs=1) as s1pool:
                Wh_aug = s1pool.tile([P, NB, HID + 2], BF16)
                onez = s1pool.tile([P, NB, 2], BF16)
                nc.vector.memset(onez[:, :, 0:1], 1.0)
                nc.vector.memset(onez[:, :, 1:2], 0.0)
                nc.vector.tensor_copy(Wh_aug[:, :, HID : HID + 2], onez[:])

                # s_i row for local nodes: psr = w1.T @ XTloc
                psr = pp.tile([1, LOC], F32, tag="aggz", bufs=1, name="psr")
                for k in range(2):
                    nc.tensor.matmul(
                        psr[:],
                        w12_sb[:, k, 0:1],
                        XTloc_sb[:, k, :],
                        start=(k == 0),
                        stop=(k == 1),
                    )
                sir = s1pool.tile([1, LOC], F32)
                nc.vector.tensor_copy(sir[:], psr[:])
                nc.gpsimd.partition_broadcast(B_sb[:], sir[:])
                me1.emit(32)

                for o in range(NB):
                    xtc = wk.tile([P, 2, P], BF16, tag="xw", bufs=12)
                    nc.scalar.dma_start(
                        xtc[:],
                        XT_d.ap()
                        .rearrange("(k p) n -> p k n", p=P)[:, :, o * P : (o + 1) * P],
                    )
                    pa = pp.tile([P, HID + 2], F32, tag="pa", bufs=2, name="pa")
                    for k in range(2):
                        nc.tensor.matmul(
                            pa[:],
                            xtc[:, k, :],
                            Ws_sb[:, k, :],
                            start=(k == 0),
                            stop=(k == 1),
                        )
                    nc.vector.tensor_copy(Wh_aug[:, o, :HID], pa[:, :HID])
                    nc.vector.tensor_copy(s_nat[:, o : o + 1], pa[:, HID + 1 :])
                    me1.emit(2)
                nc.vector.tensor_scalar(
                    s2_nat[:], s_nat[:], ALPHA, None, mybir.AluOpType.mult
                )

                # =========== phase 3: stage-1 attention ===========
                u0 = pp.tile([P, LOC], F32, tag="agg", bufs=2, name="u0")
                u1 = pp.tile([P, LOC], F32, tag="agg", bufs=2, name="u1")
                uz = pp.tile([2, LOC], F32, tag="aggz", bufs=1, name="uz")
                for jc in range(NB):
                    # exp(lrelu(e)) = max(exp(e), exp(alpha*e)) on ACT
                    e1 = wk.tile([P, LOC], BF16, tag="s1", bufs=16)
                    nc.scalar.activation(
                        e1[:], B_sb[:], mybir.ActivationFunctionType.Exp,
                        bias=s_nat[:, jc : jc + 1],
                    )
                    e2 = wk.tile([P, LOC], BF16, tag="s1", bufs=16)
                    nc.scalar.activation(
                        e2[:], B_sb[:], mybir.ActivationFunctionType.Exp,
                        bias=s2_nat[:, jc : jc + 1], scale=ALPHA,
                    )
                    mx = wk.tile([P, LOC], BF16, tag="s1", bufs=16)
                    nc.vector.tensor_max(out=mx[:], in0=e1[:], in1=e2[:])
                    em = wk.tile([P, LOC], BF16, tag="s1", bufs=16)
                    nc.vector.tensor_mul(out=em[:], in0=mx[:], in1=M0[:, jc])
                    last = jc == NB - 1
                    nc.tensor.matmul(
                        u0[:], Wh_aug[:, jc, 0:P], em[:],
                        start=(jc == 0), stop=last,
                    )
                    nc.tensor.matmul(
                        u1[:], Wh_aug[:, jc, P : 2 * P], em[:],
                        start=(jc == 0), stop=last,
                    )
                    nc.tensor.matmul(
                        uz[:], Wh_aug[:, jc, HID : HID + 2], em[:],
                        start=(jc == 0), stop=last,
                    )

                me1.emit(48)
                # normalize + gelu -> h_local.T [256, 512]
                zrow = s1pool.tile([1, LOC], F32)
                nc.vector.tensor_copy(zrow[:], uz[0:1, :])
                zb = s1pool.tile([P, LOC], F32)
                nc.gpsimd.partition_broadcast(zb[:], zrow[:])
                zr = s1pool.tile([P, LOC], F32)
                nc.vector.reciprocal_approx_fast(out=zr[:], in_=zb[:])
                for mt, um in enumerate((u0, u1)):
                    tnorm = wk.tile([P, LOC], F32, tag="nrm", bufs=3)
                    nc.vector.tensor_mul(out=tnorm[:], in0=um[:], in1=zr[:])
                    nc.scalar.activation(
                        hT[:, mt], tnorm[:], mybir.ActivationFunctionType.Gelu
                    )
                    nc.vector.tensor_copy(hTb[:, mt], hT[:, mt])

            # =========== phase 4: h transposes + gathers + WaT ===========
            nc.vector.memset(hnat[:, :, HID : HID + 1], 1.0)
            nc.vector.memset(hnat[:, :, HID + 1 : HID + 2], 0.0)
            for ic in range(LB):
                for fc in range(2):
                    pht = pp.tile([P, P], F32R, tag="pa", bufs=2, name="pht")
                    nc.tensor.transpose(
                        pht[:], hT[:, fc, ic * P : (ic + 1) * P], ident_r[:]
                    )
                    nc.vector.tensor_copy(hnat[:, ic, fc * P : (fc + 1) * P],
                                          pht[:])
            nc.scalar.dma_start(
                gat_loc.ap()[0 : LOC * (HID + 2)]
                .rearrange("(c p f) -> p c f", p=P, f=HID + 2),
                hnat[:],
            )
            # local Wa.T block = W_l.T @ h_local.T
            for m2 in range(2):
                pwa = pp.tile([P, LOC], F32, tag="pa", bufs=2, name="pwa")
                for f in range(2):
                    nc.tensor.matmul(
                        pwa[:],
                        Wl_sb[:, f, m2 * P : (m2 + 1) * P],
                        hT[:, f, :],
                        start=(f == 0),
                        stop=(f == 1),
                    )
                nc.vector.tensor_copy(WaTloc[:, m2], pwa[:])
            nc.scalar.dma_start(
                gat_loc.ap()[LOC * (HID + 2) : GATSZ]
                .rearrange("(k p n) -> p k n", p=P, n=LOC),
                WaTloc[:],
            )
            nc.gpsimd.collective_compute(
                "AllGather",
                mybir.AluOpType.bypass,
                ins=[gat_loc[:]],
                outs=[gat_all[:]],
                replica_groups=groups,
            )

            # finish mask1, then mask2 (collectives overlap this stream)
            me1.emit(512)
            me2 = MaskEmitter(M1, M2, "b", bin_engine="dve")
            me2.emit(512)

            with tc.tile_pool(name="hpool", bufs=1) as hp:
                h_aug = hp.tile([P, NB, HID + 2], BF16, name="h_aug")
                for c in range(NCORES):
                    nc.scalar.dma_start(
                        h_aug[:, LB * c : LB * (c + 1)],
                        gat_all.ap()[c * GATSZ : c * GATSZ + LOC * (HID + 2)]
                        .rearrange("(c2 p f) -> p c2 f", p=P, f=HID + 2),
                    )
                expS = hp.tile([P, NB, LOC], BF16, name="expS")

                # ---- scores + expS (needs WaT gather) ----
                with tc.tile_pool(name="scpool", bufs=1) as scpool:
                    WaTall = scpool.tile([P, 2 * NCORES, LOC], BF16)
                    for c in range(NCORES):
                        nc.scalar.dma_start(
                            WaTall[:, 2 * c : 2 * (c + 1)],
                            gat_all.ap()[c * GATSZ + LOC * (HID + 2)
                                         : (c + 1) * GATSZ]
                            .rearrange("(k p n) -> p k n", p=P, n=LOC),
                        )
                    for m in range(NB):
                        pst = pp.tile([P, LOC], F32, tag="pa", bufs=2, name="pst")
                        c, mi = divmod(m, LB)
                        for f in range(2):
                            nc.tensor.matmul(
                                pst[:],
                                WaTall[:, 2 * c + f, mi * P : (mi + 1) * P],
                                hTb[:, f, :],
                                start=(f == 0),
                                stop=(f == 1),
                            )
                        nc.scalar.activation(
                            expS[:, m], pst[:], mybir.ActivationFunctionType.Exp
                        )

                # =========== hops ===========
                def hop(mask_fp8, first, tags=("agg", "aggz")):
                    u0h = pp.tile([P, LOC], F32, tag=tags[0], bufs=2, name="u0h")
                    u1h = pp.tile([P, LOC], F32, tag=tags[0], bufs=2, name="u1h")
                    uzh = pp.tile([2, LOC], F32, tag=tags[1], bufs=1, name="uzh")
                    for m in range(NB):
                        ek = wk.tile([P, LOC], BF16, tag="ek", bufs=6)
                        nc.vector.tensor_mul(
                            out=ek[:], in0=expS[:, m], in1=mask_fp8[:, m]
                        )
                        last = m == NB - 1
                        nc.tensor.matmul(
                            u0h[:], h_aug[:, m, 0:P], ek[:],
                            start=(m == 0), stop=last,
                        )
                        nc.tensor.matmul(
                            u1h[:], h_aug[:, m, P : 2 * P], ek[:],
                            start=(m == 0), stop=last,
                        )
                        nc.tensor.matmul(
                            uzh[:], h_aug[:, m, HID : HID + 2], ek[:],
                            start=(m == 0), stop=last,
                        )
                    zrowh = wk.tile([1, LOC], F32, tag="row", bufs=2)
                    nc.vector.tensor_copy(zrowh[:], uzh[0:1, :])
                    zbh = wk.tile([P, LOC], F32, tag="nrm", bufs=3)
                    nc.gpsimd.partition_broadcast(zbh[:], zrowh[:])
                    zrh = wk.tile([P, LOC], F32, tag="nrm", bufs=3)
                    nc.vector.reciprocal_approx_fast(out=zrh[:], in_=zbh[:])
                    for mt, um in enumerate((u0h, u1h)):
                        tn = wk.tile([P, LOC], F32R, tag="nrm", bufs=3)
                        nc.vector.tensor_mul(out=tn[:], in0=um[:], in1=zrh[:])
                        if first:
                            nc.vector.tensor_add(
                                out=outT[:, mt], in0=hT[:, mt], in1=tn[:]
                            )
                        else:
                            nc.vector.tensor_add(
                                out=outT[:, mt], in0=outT[:, mt], in1=tn[:]
                            )

                hop(M0, first=True)
                hop(M1, first=False, tags=("pa", "maskB"))
                hop(M2, first=False)

            # =========== output projection ===========
            py = pp.tile([P, LOC], F32, tag="pa", bufs=2, name="py")
            for k in range(2):
                nc.tensor.matmul(
                    py[:],
                    Wo_sb[:, k, :],
                    outT[:, k, :],
                    start=(k == 0),
                    stop=(k == 1),
                )
            yt = sm.tile([P, LOC], F32, name="yt")
            nc.vector.tensor_scalar(
                yt[:], py[:], bo_sb[:, 0:1], None, mybir.AluOpType.add
            )
            nc.scalar.dma_start(out_d[:, :], yt[:])

    nc.compile()
    return nc


def _get_nc():
    if "nc" not in _CACHE:
        _CACHE["nc"] = build_kernel()
    return _CACHE["nc"]


def kernel(X, A, W_s, r, W_l, W_out, b_out):
    global last_in_maps
    import ml_dtypes

    FP8NP = ml_dtypes.float8_e4m3

    X = np.ascontiguousarray(X, dtype=np.float32)
    A = np.ascontiguousarray(A, dtype=np.float32)
    W_s = np.ascontiguousarray(W_s, dtype=np.float32)
    r = np.ascontiguousarray(r, dtype=np.float32)

    import ml_dtypes as _mld

    XTf = np.ascontiguousarray(X.T)                      # [HID, N] f32
    XT = XTf.astype(_mld.bfloat16)                       # [HID, N] bf16
    A8 = A.astype(FP8NP)                                 # [N, N] (0/1, exact)
    AT8 = np.ascontiguousarray(A8.T)                     # [N, N]
    w1 = W_s @ r[:HID]                                   # [HID, 1]
    w2 = W_s @ r[HID:]                                   # [HID, 1]
    w12 = np.ascontiguousarray(
        np.concatenate([w1, w2], axis=1), dtype=np.float32
    )                                                    # [HID, 2]
    Ws_aug = np.ascontiguousarray(
        np.concatenate([W_s, w1, w2], axis=1)
    ).astype(_mld.bfloat16)                              # [HID, HID+2] bf16

    in_maps = []
    for c in range(NCORES):
        sl = slice(c * LOC, (c + 1) * LOC)
        in_maps.append(
            {
                "XT": XT,
                "XTloc": np.ascontiguousarray(XTf[:, sl]),
                "A8": A8,
                "AT8": np.ascontiguousarray(AT8[:, sl]),
                "Ws_aug": Ws_aug,
                "w12": w12,
                "W_l": np.ascontiguousarray(W_l, dtype=np.float32),
                "W_out": np.ascontiguousarray(W_out, dtype=np.float32),
                "b_out": np.ascontiguousarray(b_out, dtype=np.float32),
            }
        )
    last_in_maps = in_maps
    nc = _get_nc()
    res = run_bass_kernel_spmd(nc, in_maps, core_ids=list(range(NCORES)))
    Y = np.empty((N, OUT_DIM), dtype=np.float32)
    for c in range(NCORES):
        Y[c * LOC : (c + 1) * LOC, :] = res.results[c]["out"].T
    return Y


if __name__ == "__main__":
    build_kernel()
    print("build OK")


# revision 17
# speedup vs baseline: 1.1290x; 1.0013x over previous
"""Trainium2 Bass kernel for nn_LongDistanceAttention (GNN message passing).

Strategy (8 NeuronCores, SPMD, node/row sharding). v2:
  Host prep: A cast to fp8 once (A8 natural, AT8 = per-core A.T column
  block = 1-hop mask M0), X pre-transposed (XT full, XTloc per-core),
  W_s augmented with w1 = W_s@r[:H], w2 = W_s@r[H:] columns. This removes
  the on-device A fp8-cast pipeline, the A8 AllGather (125us unoverlapped
  in v1), and all X/W PE transposes.

  Device, all N x N work on transposed layout [j(source) x i(local rows)]:
    - phase 2: Wh_aug rows + s_j scalars in ONE matmul per chunk against
      the augmented weight; s_i row via w1-column matmul on XTloc.
    - stage 1 GAT: exp(lrelu(s_i+s_j)) = max(exp(e), exp(0.2e)) -> two
      ACT exps with per-partition bias, max + mask-mul on DVE;
      (E @ [Wh | 1 | 0]).T accumulated on PE gives numerator and row-sum.
    - k-hop masks: A^k via fp8 DoubleRow matmuls (exact: 0/1 inputs,
      fp32 PSUM accumulation), binarized by ACT Sign. The 2x512 DR
      instruction stream is interleaved into phase-2/stage-1 PE idle
      slots via MaskEmitter so the PE never drains.
    - h (bf16, ones column) and WaT blocks (f32) all-gathered; both
      collectives overlap the mask2 matmul stream.
    - per hop: ek = expS * mask_k (bf16*fp8 on DVE); U.T/Z via PE;
      normalization via broadcast-then-reciprocal (partition-parallel).
  Final: Y.T = W_out.T @ out.T + b_out, output per core [128, 512].
"""

import sys

import numpy as np

sys.path.insert(0, "/opt/trn_rl_repo")

import concourse.bass as bass  # noqa: E402
import concourse.mybir as mybir  # noqa: E402
import concourse.tile as tile  # noqa: E402
from concourse import bacc  # noqa: E402
from concourse.bass_utils import run_bass_kernel_spmd  # noqa: E402
from concourse.masks import make_identity  # noqa: E402

P = 128
N = 4096
NB = N // P            # 32 j-chunks
HID = 256
OUT_DIM = 128
NCORES = 8
LOC = N // NCORES      # 512 local rows per core
LB = LOC // P          # 4 local partition chunks
ALPHA = 0.2

F32 = mybir.dt.float32
F32R = mybir.dt.float32r
BF16 = mybir.dt.bfloat16
FP8 = mybir.dt.float8e4

_CACHE = {}
last_in_maps = None


def build_kernel():
    nc = bacc.Bacc(
        "TRN2",
        target_bir_lowering=False,
        debug=False,
        enable_asserts=False,
        num_devices=NCORES,
    )

    # ---- kernel I/O (host-prepped layouts) ----
    XT_d = nc.dram_tensor("XT", [HID, N], BF16, kind="ExternalInput")
    XTloc_d = nc.dram_tensor("XTloc", [HID, LOC], F32, kind="ExternalInput")
    A8_d = nc.dram_tensor("A8", [N, N], FP8, kind="ExternalInput")
    AT8_d = nc.dram_tensor("AT8", [N, LOC], FP8, kind="ExternalInput")
    ATb_d = nc.dram_tensor("ATb", [N, LOC], BF16, kind="ExternalInput")
    Wsa_d = nc.dram_tensor("Ws_aug", [HID, HID + 2], BF16, kind="ExternalInput")
    w12_d = nc.dram_tensor("w12", [HID, 2], F32, kind="ExternalInput")
    Wl_d = nc.dram_tensor("W_l", [HID, HID], F32, kind="ExternalInput")
    Wo_d = nc.dram_tensor("W_out", [HID, OUT_DIM], F32, kind="ExternalInput")
    bo_d = nc.dram_tensor("b_out", [OUT_DIM], F32, kind="ExternalInput")
    out_d = nc.dram_tensor("out", [OUT_DIM, LOC], F32, kind="ExternalOutput")

    # ---- internal DRAM (single gather blob: hnat bf16 ++ WaT bf16) ----
    wat_loc = nc.dram_tensor("wat_loc", [HID, LOC], BF16)
    wat_all = nc.dram_tensor("wat_all", [HID * NCORES, LOC], BF16,
                             addr_space="Shared")
    haug_loc = nc.dram_tensor("haug_loc", [LOC, HID + 2], BF16)
    haug_all = nc.dram_tensor("haug_all", [N, HID + 2], BF16,
                              addr_space="Shared")

    groups = [list(range(NCORES))]

    with tile.TileContext(nc) as tc:
        with (
            tc.tile_pool(name="const", bufs=1) as cpool,
            tc.tile_pool(name="small", bufs=1) as sm,
            tc.tile_pool(name="maskp", bufs=1) as mp,
            tc.tile_pool(name="wk", bufs=1) as wk,
            tc.tile_pool(name="pp", bufs=1, space="PSUM") as pp,
        ):
            # =========== constants / weights / masks (ACT queue) ===========
            # M0 first (mask stream feeds on it immediately), in 4 chunks.
            M0 = mp.tile([P, NB, LOC], FP8, name="M0")
            at8_r = AT8_d.ap().rearrange("(c p) n -> p c n", p=P)
            nc.scalar.dma_start(M0[:, 0:2], at8_r[:, 0:2])
            nc.scalar.dma_start(M0[:, 2:8], at8_r[:, 2:8])
            XTloc_sb = cpool.tile([P, 2, LOC], F32R)
            nc.scalar.dma_start(
                XTloc_sb[:],
                XTloc_d.ap().rearrange("(k p) n -> p k n", p=P).bitcast(F32R),
            )
            for q in range(1, 4):
                nc.scalar.dma_start(M0[:, 8 * q : 8 * (q + 1)],
                                    at8_r[:, 8 * q : 8 * (q + 1)])
            Ws_sb = cpool.tile([P, 2, HID + 2], BF16)
            nc.scalar.dma_start(
                Ws_sb[:], Wsa_d.ap().rearrange("(k p) m -> p k m", p=P)
            )
            w12_sb = cpool.tile([P, 2, 2], F32R)
            nc.scalar.dma_start(
                w12_sb[:],
                w12_d.ap().rearrange("(k p) m -> p k m", p=P).bitcast(F32R),
            )
            Wl_sb = cpool.tile([P, 2, HID], F32R)
            nc.scalar.dma_start(
                Wl_sb[:], Wl_d.ap().rearrange("(k p) m -> p k m", p=P).bitcast(F32R)
            )
            Wo_sb = cpool.tile([P, 2, OUT_DIM], F32R)
            nc.scalar.dma_start(
                Wo_sb[:], Wo_d.ap().rearrange("(k p) m -> p k m", p=P).bitcast(F32R)
            )
            bo_sb = cpool.tile([P, 1], F32)
            nc.scalar.dma_start(bo_sb[:], bo_d.ap().rearrange("(o p) -> p o", p=P))
            M0b = mp.tile([P, NB, LOC], BF16, name="M0b")
            atb_r = ATb_d.ap().rearrange("(c p) n -> p c n", p=P)
            for q in range(4):
                nc.scalar.dma_start(M0b[:, 8 * q : 8 * (q + 1)],
                                    atb_r[:, 8 * q : 8 * (q + 1)])
            ident = cpool.tile([P, P], F32)
            make_identity(nc, ident)
            ident_r = cpool.tile([P, P], F32R)
            nc.vector.tensor_copy(ident_r[:], ident[:])
            ones_f = cpool.tile([1, P], F32)
            nc.vector.memset(ones_f[:], 1.0)
            ones_row = cpool.tile([1, P], F32R)
            nc.vector.tensor_copy(ones_row[:], ones_f[:])

            # masks (persist across hops)
            M1 = mp.tile([P, NB, LOC], FP8, name="M1")
            M2 = mp.tile([P, NB, LOC], FP8, name="M2")

            # small persistent tiles
            hT = sm.tile([P, 2, LOC], F32R, name="hT")
            hnat = sm.tile([P, LB, HID + 2], BF16, name="hnat")
            WaTloc = sm.tile([P, 2, LOC], BF16, name="WaTloc")
            s_nat = sm.tile([P, NB], F32, name="s_nat")
            s2_nat = sm.tile([P, NB], F32, name="s2_nat")
            B_sb = sm.tile([P, LOC], F32, name="B_sb")
            hTb = sm.tile([P, 2, LOC], BF16, name="hTb")

            # =========== mask matmul emitter (A^k via fp8 DR) ===========
            a8_r = A8_d.ap().rearrange("(kq ko p) n -> p ko kq n", p=P, ko=8)

            class MaskEmitter:
                """Emits the A.T @ rhs fp8-DoubleRow stream (512 matmuls)
                in resumable slabs so mask matmuls fill PE gaps in other
                phases. Per mg (16): kq(4) x s(4) x mi(2) = 32 matmuls,
                then binarize the two PSUM tiles ("act" Sign / "dve"
                is_gt) into the out mask columns."""

                def __init__(self, rhs_tile, out_tile, tag, bin_engine):
                    self.rhs = rhs_tile
                    self.out = out_tile
                    self.tag = tag
                    self.bin_engine = bin_engine
                    self.pos = 0          # 0..511
                    self.pms = None
                    self.a8t = None

                def emit(self, n):
                    end = min(self.pos + n, 512)
                    while self.pos < end:
                        idx = self.pos
                        mg, r = divmod(idx, 32)
                        kq, r2 = divmod(r, 8)
                        s, mi = divmod(r2, 2)
                        if r == 0:
                            self.pms = [
                                pp.tile([P, LOC], F32, tag="mask", bufs=2,
                                        name=f"pm{self.tag}0"),
                                pp.tile([P, LOC], F32, tag="maskB", bufs=1,
                                        name=f"pm{self.tag}1"),
                            ]
                        if r2 == 0:
                            # one DMA per (mg, kq): 8 k-chunks x 256 cols
                            self.a8t = wk.tile([P, 8, 2 * P], FP8, tag="a8t",
                                               bufs=4)
                            nc.sync.dma_start(
                                self.a8t[:],
                                a8_r[:, :, kq, 2 * P * mg : 2 * P * (mg + 1)],
                            )
                        nc.tensor.matmul(
                            self.pms[mi][:],
                            self.a8t[:, 2 * s : 2 * s + 2,
                                     mi * P : (mi + 1) * P],
                            self.rhs[:, 8 * kq + 2 * s : 8 * kq + 2 * s + 2, :],
                            start=(kq == 0 and s == 0),
                            stop=(kq == 3 and s == 3),
                            perf_mode=mybir.MatmulPerfMode.DoubleRow,
                        )
                        if r == 31:
                            for m2 in range(2):
                                if self.bin_engine == "act":
                                    nc.scalar.activation(
                                        self.out[:, 2 * mg + m2],
                                        self.pms[m2][:],
                                        mybir.ActivationFunctionType.Sign,
                                    )
                                else:
                                    nc.vector.tensor_scalar(
                                        self.out[:, 2 * mg + m2],
                                        self.pms[m2][:],
                                        0.5,
                                        None,
                                        mybir.AluOpType.is_gt,
                                    )
                        self.pos += 1

            me1 = MaskEmitter(M0, M1, "a", bin_engine="dve")

            # =========== phase 2: Wh_aug + s vectors ===========
            with tc.tile_pool(name="s1pool", bufs=1) as s1pool:
                Wh_aug = s1pool.tile([P, NB, HID + 2], BF16)
                onez = s1pool.tile([P, NB, 2], BF16)
                nc.vector.memset(onez[:, :, 0:1], 1.0)
                nc.vector.memset(onez[:, :, 1:2], 0.0)
                nc.vector.tensor_copy(Wh_aug[:, :, HID : HID + 2], onez[:])

                # s_i row for local nodes: psr = w1.T @ XTloc
                psr = pp.tile([1, LOC], F32, tag="aggz", bufs=1, name="psr")
                for k in range(2):
                    nc.tensor.matmul(
                        psr[:],
                        w12_sb[:, k, 0:1],
                        XTloc_sb[:, k, :],
                        start=(k == 0),
                        stop=(k == 1),
                    )
                sir = s1pool.tile([1, LOC], F32)
                nc.vector.tensor_copy(sir[:], psr[:])
                nc.gpsimd.partition_broadcast(B_sb[:], sir[:])
                me1.emit(32)

                for o in range(NB):
                    xtc = wk.tile([P, 2, P], BF16, tag="xw", bufs=12)
                    nc.scalar.dma_start(
                        xtc[:],
                        XT_d.ap()
                        .rearrange("(k p) n -> p k n", p=P)[:, :, o * P : (o + 1) * P],
                    )
                    pa = pp.tile([P, HID + 2], F32, tag="pa", bufs=2, name="pa")
                    for k in range(2):
                        nc.tensor.matmul(
                            pa[:],
                            xtc[:, k, :],
                            Ws_sb[:, k, :],
                            start=(k == 0),
                            stop=(k == 1),
                        )
                    nc.vector.tensor_copy(Wh_aug[:, o, :HID], pa[:, :HID])
                    nc.vector.tensor_copy(s_nat[:, o : o + 1], pa[:, HID + 1 :])
                    me1.emit(2)
                nc.vector.tensor_scalar(
                    s2_nat[:], s_nat[:], ALPHA, None, mybir.AluOpType.mult
                )

                # =========== phase 3: stage-1 attention ===========
                u0 = pp.tile([P, LOC], F32, tag="agg", bufs=2, name="u0")
                u1 = pp.tile([P, LOC], F32, tag="agg", bufs=2, name="u1")
                uz = pp.tile([2, LOC], F32, tag="aggz", bufs=1, name="uz")
                for jc in range(NB):
                    # exp(lrelu(e)) = max(exp(e), exp(alpha*e)) on ACT
                    e1 = wk.tile([P, LOC], BF16, tag="s1", bufs=8)
                    nc.scalar.activation(
                        e1[:], B_sb[:], mybir.ActivationFunctionType.Exp,
                        bias=s_nat[:, jc : jc + 1],
                    )
                    e2 = wk.tile([P, LOC], BF16, tag="s1", bufs=8)
                    nc.scalar.activation(
                        e2[:], B_sb[:], mybir.ActivationFunctionType.Exp,
                        bias=s2_nat[:, jc : jc + 1], scale=ALPHA,
                    )
                    mx = wk.tile([P, LOC], BF16, tag="s1", bufs=8)
                    nc.vector.tensor_max(out=mx[:], in0=e1[:], in1=e2[:])
                    em = wk.tile([P, LOC], BF16, tag="s1", bufs=8)
                    nc.vector.tensor_mul(out=em[:], in0=mx[:], in1=M0b[:, jc])
                    last = jc == NB - 1
                    nc.tensor.matmul(
                        u0[:], Wh_aug[:, jc, 0:P], em[:],
                        start=(jc == 0), stop=last,
                    )
                    nc.tensor.matmul(
                        u1[:], Wh_aug[:, jc, P : 2 * P], em[:],
                        start=(jc == 0), stop=last,
                    )
                    nc.tensor.matmul(
                        uz[:], Wh_aug[:, jc, HID : HID + 2], em[:],
                        start=(jc == 0), stop=last,
                    )

                me1.emit(48)
                # normalize + gelu -> h_local.T [256, 512]
                zrow = s1pool.tile([1, LOC], F32)
                nc.vector.tensor_copy(zrow[:], uz[0:1, :])
                zb = s1pool.tile([P, LOC], F32)
                nc.gpsimd.partition_broadcast(zb[:], zrow[:])
                zr = s1pool.tile([P, LOC], F32)
                nc.vector.reciprocal_approx_fast(out=zr[:], in_=zb[:])
                for mt, um in enumerate((u0, u1)):
                    tnorm = wk.tile([P, LOC], F32, tag="nrm", bufs=3)
                    nc.vector.tensor_mul(out=tnorm[:], in0=um[:], in1=zr[:])
                    nc.scalar.activation(
                        hT[:, mt], tnorm[:], mybir.ActivationFunctionType.Gelu
                    )
                    nc.vector.tensor_copy(hTb[:, mt], hT[:, mt])

            # =========== phase 4: WaT + gather first, then h transposes ====
            # local Wa.T block = W_l.T @ h_local.T
            for m2 in range(2):
                pwa = pp.tile([P, LOC], F32, tag="pa", bufs=2, name="pwa")
                for f in range(2):
                    nc.tensor.matmul(
                        pwa[:],
                        Wl_sb[:, f, m2 * P : (m2 + 1) * P],
                        hT[:, f, :],
                        start=(f == 0),
                        stop=(f == 1),
                    )
                nc.vector.tensor_copy(WaTloc[:, m2], pwa[:])
            nc.scalar.dma_start(
                wat_loc.ap().rearrange("(c p) n -> p c n", p=P), WaTloc[:]
            )
            nc.gpsimd.collective_compute(
                "AllGather",
                mybir.AluOpType.bypass,
                ins=[wat_loc[:, :]],
                outs=[wat_all[:, :]],
                replica_groups=groups,
            )
            nc.vector.memset(hnat[:, :, HID : HID + 1], 1.0)
            nc.vector.memset(hnat[:, :, HID + 1 : HID + 2], 0.0)
            for ic in range(LB):
                for fc in range(2):
                    pht = pp.tile([P, P], F32R, tag="pa", bufs=2, name="pht")
                    nc.tensor.transpose(
                        pht[:], hT[:, fc, ic * P : (ic + 1) * P], ident_r[:]
                    )
                    nc.vector.tensor_copy(hnat[:, ic, fc * P : (fc + 1) * P],
                                          pht[:])
            nc.scalar.dma_start(
                haug_loc.ap().rearrange("(c p) f -> p c f", p=P), hnat[:]
            )
            nc.gpsimd.collective_compute(
                "AllGather",
                mybir.AluOpType.bypass,
                ins=[haug_loc[:, :]],
                outs=[haug_all[:, :]],
                replica_groups=groups,
            )

            # finish mask1, then mask2 (collectives overlap this stream)
            me1.emit(512)
            me2 = MaskEmitter(M1, M2, "b", bin_engine="dve")
            me2.emit(512)

            with tc.tile_pool(name="hpool", bufs=1) as hp:
                h_aug = hp.tile([P, NB, HID + 2], BF16, name="h_aug")
                nc.scalar.dma_start(
                    h_aug[:], haug_all.ap().rearrange("(o p) f -> p o f", p=P)
                )
                expS = hp.tile([P, NB, LOC], BF16, name="expS")

                # ---- scores + expS (needs WaT gather) ----
                with tc.tile_pool(name="scpool", bufs=1) as scpool:
                    WaTall = scpool.tile([P, 2 * NCORES, LOC], BF16)
                    nc.scalar.dma_start(
                        WaTall[:],
                        wat_all.ap().rearrange("(o p) n -> p o n", p=P),
                    )
                    for m in range(NB):
                        pst = pp.tile([P, LOC], F32, tag="pa", bufs=2, name="pst")
                        c, mi = divmod(m, LB)
                        for f in range(2):
                            nc.tensor.matmul(
                                pst[:],
                                WaTall[:, 2 * c + f, mi * P : (mi + 1) * P],
                                hTb[:, f, :],
                                start=(f == 0),
                                stop=(f == 1),
                            )
                        nc.scalar.activation(
                            expS[:, m], pst[:], mybir.ActivationFunctionType.Exp
                        )

                # =========== hops ===========
                def hop(mask_fp8, tags=("agg", "aggz"), last_hop=False):
                    u0h = pp.tile([P, LOC], F32, tag=tags[0], bufs=2, name="u0h")
                    u1h = pp.tile([P, LOC], F32, tag=tags[0], bufs=2, name="u1h")
                    uzh = pp.tile([2, LOC], F32, tag=tags[1], bufs=1, name="uzh")
                    for m in range(NB):
                        ek = wk.tile([P, LOC], BF16, tag="ek", bufs=6)
                        nc.vector.tensor_mul(
                            out=ek[:], in0=expS[:, m], in1=mask_fp8[:, m]
                        )
                        last = m == NB - 1
                        nc.tensor.matmul(
                            u0h[:], h_aug[:, m, 0:P], ek[:],
                            start=(m == 0), stop=last,
                        )
                        nc.tensor.matmul(
                            u1h[:], h_aug[:, m, P : 2 * P], ek[:],
                            start=(m == 0), stop=last,
                        )
                        nc.tensor.matmul(
                            uzh[:], h_aug[:, m, HID : HID + 2], ek[:],
                            start=(m == 0), stop=last,
                        )
                    zrowh = wk.tile([1, LOC], F32, tag="row", bufs=2)
                    nc.vector.tensor_copy(zrowh[:], uzh[0:1, :])
                    zbh = wk.tile([P, LOC], F32, tag="nrm", bufs=3)
                    nc.gpsimd.partition_broadcast(zbh[:], zrowh[:])
                    zrh = wk.tile([P, LOC], F32, tag="nrm", bufs=3)
                    nc.vector.reciprocal_approx_fast(out=zrh[:], in_=zbh[:])
                    for mt, um in enumerate((u0h, u1h)):
                        tn = wk.tile([P, LOC], F32R, tag="nrm", bufs=3)
                        nc.vector.tensor_mul(out=tn[:], in0=um[:], in1=zrh[:])
                        nc.tensor.matmul(
                            py[:], Wo_sb[:, mt, :], tn[:],
                            start=False, stop=(last_hop and mt == 1),
                        )

                # Y.T accumulated in PSUM: Wo.T @ (hT + sum_k tn_k)
                py = pp.tile([P, LOC], F32, tag="mask", bufs=2, name="py")
                for k in range(2):
                    nc.tensor.matmul(
                        py[:], Wo_sb[:, k, :], hT[:, k, :],
                        start=(k == 0), stop=False,
                    )
                hop(M0b)
                hop(M1, tags=("pa", "maskB"))
                hop(M2, last_hop=True)

            # =========== output: bias + store ===========
            yt = sm.tile([P, LOC], F32, name="yt")
            nc.vector.tensor_scalar(
                yt[:], py[:], bo_sb[:, 0:1], None, mybir.AluOpType.add
            )
            nc.scalar.dma_start(out_d[:, :], yt[:])

    nc.compile()
    return nc


def _get_nc():
    if "nc" not in _CACHE:
        _CACHE["nc"] = build_kernel()
    return _CACHE["nc"]


def kernel(X, A, W_s, r, W_l, W_out, b_out):
    global last_in_maps
    import ml_dtypes

    FP8NP = ml_dtypes.float8_e4m3

    X = np.ascontiguousarray(X, dtype=np.float32)
    A = np.ascontiguousarray(A, dtype=np.float32)
    W_s = np.ascontiguousarray(W_s, dtype=np.float32)
    r = np.ascontiguousarray(r, dtype=np.float32)

    import ml_dtypes as _mld

    XTf = np.ascontiguousarray(X.T)                      # [HID, N] f32
    XT = XTf.astype(_mld.bfloat16)                       # [HID, N] bf16
    A8 = A.astype(FP8NP)                                 # [N, N] (0/1, exact)
    AT8 = np.ascontiguousarray(A8.T)                     # [N, N]
    w1 = W_s @ r[:HID]                                   # [HID, 1]
    w2 = W_s @ r[HID:]                                   # [HID, 1]
    w12 = np.ascontiguousarray(
        np.concatenate([w1, w2], axis=1), dtype=np.float32
    )                                                    # [HID, 2]
    Ws_aug = np.ascontiguousarray(
        np.concatenate([W_s, w1, w2], axis=1)
    ).astype(_mld.bfloat16)                              # [HID, HID+2] bf16

    in_maps = []
    for c in range(NCORES):
        sl = slice(c * LOC, (c + 1) * LOC)
        in_maps.append(
            {
                "XT": XT,
                "XTloc": np.ascontiguousarray(XTf[:, sl]),
                "A8": A8,
                "AT8": np.ascontiguousarray(AT8[:, sl]),
                "ATb": np.ascontiguousarray(A.T[:, sl]).astype(_mld.bfloat16),
                "Ws_aug": Ws_aug,
                "w12": w12,
                "W_l": np.ascontiguousarray(W_l, dtype=np.float32),
                "W_out": np.ascontiguousarray(W_out, dtype=np.float32),
                "b_out": np.ascontiguousarray(b_out, dtype=np.float32),
            }
        )
    last_in_maps = in_maps
    nc = _get_nc()
    res = run_bass_kernel_spmd(nc, in_maps, core_ids=list(range(NCORES)))
    Y = np.empty((N, OUT_DIM), dtype=np.float32)
    for c in range(NCORES):
        Y[c * LOC : (c + 1) * LOC, :] = res.results[c]["out"].T
    return Y


if __name__ == "__main__":
    build_kernel()
    print("build OK")
